# revision 1
# baseline (speedup 1.0000x reference)
"""Distributed single-head transformer block on 8 TRN2 NeuronCores.

Sharding: token dim (4096) split 8 ways (512 tokens/core). Weights are
replicated (host pre-transposes them so every matmul contracts over the
partition axis with zero on-chip transposes). Attention needs all tokens'
K/V, so each core computes its local K^T and V, converts to bf16, and two
AllGathers (K^T first, then V) distribute them while the PE keeps
computing (K^T gather overlaps q/V projections; V gather overlaps the
score phase). All other compute is local to the core's 512 tokens,
operating feature-major ("T-domain": [feature, token] layout):

  qT/kT = WT.T @ xT            (fp32r matmuls, fp32 PSUM accumulate)
  S_r^T = kT_r.T @ qT -> exp   (scores arrive transposed; softmax sum over
  denom = ones.T @ exp(S^T)     the partition axis via a ones-matmul,
                                interleaved with the score matmuls)
  attnT = V.T @ P^T             (V gathered token-major is exactly lhsT)
  LN in T-domain: mean/var via ones-matmuls, per-token broadcast via DRAM
  FFN chunked over the hidden dim so the intermediate stays small.

Output is outT [D, 512] per core; the host transposes and concatenates.
"""

import numpy as np

P = 128
D = 1024
N = 4096
H = 4096
NCORES = 8
TOK = N // NCORES  # 512 tokens per core
DK = D // P  # 8   feature k/m-tiles
MT = TOK // P  # 4   local token tiles
NJ = N // P  # 32  global token k-tiles
HM = H // P  # 32  hidden m-tiles
HC = 4  # FFN hidden chunks (H / HC = 1024 per chunk)
HCK = H // HC // P  # 8 k-tiles per hidden chunk
SCALE = 1.0 / float(np.sqrt(D))
EXPBIAS = 3.0  # softmax exp bias so fp8 probs stay in e4m3 normal range
EPS = 1e-5
KV_K = D * TOK  # elements of the kT gather buffer per rank
KV_V = TOK * D  # elements of the V gather buffer per rank

_cache = {}


def _build_nc():
    import concourse.tile as tile
    from concourse import bacc, mybir
    from contextlib import ExitStack

    f32 = mybir.dt.float32
    f32r = mybir.dt.float32r
    bf16 = mybir.dt.bfloat16
    f8 = mybir.dt.float8e4
    Exp = mybir.ActivationFunctionType.Exp
    Sqrt = mybir.ActivationFunctionType.Sqrt
    mult = mybir.AluOpType.mult
    add = mybir.AluOpType.add

    nc = bacc.Bacc("TRN2", target_bir_lowering=False, debug=False, num_devices=NCORES)

    xT = nc.dram_tensor("xT", [D, TOK], f32, kind="ExternalInput").ap()
    WqT = nc.dram_tensor("WqT", [D, D], bf16, kind="ExternalInput").ap()
    WkT = nc.dram_tensor("WkT", [D, D], bf16, kind="ExternalInput").ap()
    WvT = nc.dram_tensor("WvT", [D, D], bf16, kind="ExternalInput").ap()
    W1T = nc.dram_tensor("W1T", [D, H], bf16, kind="ExternalInput").ap()
    W2T = nc.dram_tensor("W2T", [H, D], bf16, kind="ExternalInput").ap()
    bv = nc.dram_tensor("bv", [D], f32, kind="ExternalInput").ap()
    g0 = nc.dram_tensor("g0", [D], f32, kind="ExternalInput").ap()
    b0 = nc.dram_tensor("b0", [D], f32, kind="ExternalInput").ap()
    b1 = nc.dram_tensor("b1", [H], f32, kind="ExternalInput").ap()
    w2w1n = nc.dram_tensor("w2w1n", [D], f32, kind="ExternalInput").ap()
    b2 = nc.dram_tensor("b2", [D], f32, kind="ExternalInput").ap()
    g1 = nc.dram_tensor("g1", [D], f32, kind="ExternalInput").ap()
    b1n = nc.dram_tensor("b1n", [D], f32, kind="ExternalInput").ap()
    outT = nc.dram_tensor("outT", [D, TOK], f32, kind="ExternalOutput").ap()

    with tile.TileContext(nc) as tc, ExitStack() as ctx:
        dram = ctx.enter_context(tc.tile_pool(name="dram", bufs=1, space="DRAM"))
        consts = ctx.enter_context(tc.tile_pool(name="consts", bufs=1))
        xq = ctx.enter_context(tc.tile_pool(name="xq", bufs=1))
        mid = ctx.enter_context(tc.tile_pool(name="mid", bufs=2))
        big = ctx.enter_context(tc.tile_pool(name="big", bufs=1))
        wst = ctx.enter_context(tc.tile_pool(name="wst", bufs=3))
        wv_st = ctx.enter_context(tc.tile_pool(name="wv_st", bufs=2))
        kvst = ctx.enter_context(tc.tile_pool(name="kvst", bufs=2))
        vtst = ctx.enter_context(tc.tile_pool(name="vtst", bufs=3))
        ev = ctx.enter_context(tc.tile_pool(name="ev", bufs=3))
        fts = ctx.enter_context(tc.tile_pool(name="fts", bufs=1))
        ps = ctx.enter_context(tc.tile_pool(name="ps", bufs=5, space="PSUM"))
        pss = ctx.enter_context(tc.tile_pool(name="pss", bufs=2, space="PSUM"))

        KCH = 2  # K-gather chunks
        CTOK = TOK // KCH  # tokens per chunk
        KC = D * CTOK  # elements per kT token-chunk
        kv_in_k = [
            dram.tile([KC], f8, name=f"kv_in_k{c}", tag=f"kv_in_k{c}")
            for c in range(KCH)
        ]
        kv_out_k = [
            dram.tile(
                [NCORES * KC],
                f8,
                addr_space="Shared",
                name=f"kv_out_k{c}",
                tag=f"kv_out_k{c}",
            )
            for c in range(KCH)
        ]
        VCH = 2  # V gather chunks (feature halves)
        VC = (DK // VCH) * P * MT * P  # elements per V chunk per rank
        kv_in_v = [
            dram.tile([VC], f8, name=f"kv_in_v{c}", tag=f"kv_in_v{c}")
            for c in range(VCH)
        ]
        kv_out_v = [
            dram.tile(
                [NCORES * VC],
                f8,
                addr_space="Shared",
                name=f"kv_out_v{c}",
                tag=f"kv_out_v{c}",
            )
            for c in range(VCH)
        ]
        ln_dram = dram.tile([6, TOK], f32)

        # ---- constants -------------------------------------------------
        ones_f32 = consts.tile([P, 1], f32)
        nc.vector.memset(ones_f32, 1.0)
        ones_f = consts.tile([P, 1], f32r)
        nc.vector.tensor_copy(ones_f, ones_f32)
        eps_sb = consts.tile([1, 1], f32)
        nc.vector.memset(eps_sb, EPS)
        bias3_sb = consts.tile([P, 1], f32)
        nc.vector.memset(bias3_sb, EXPBIAS)
        ones_b = consts.tile([P, 1], bf16)
        nc.vector.memset(ones_b, 1.0)
        bv_b = consts.tile([P, D], f32)
        nc.gpsimd.dma_start(out=bv_b, in_=bv[None, :].to_broadcast([P, D]))
        g0_sb = consts.tile([P, DK], f32)
        nc.sync.dma_start(out=g0_sb, in_=g0.rearrange("(m p) -> p m", p=P))
        b0_sb = consts.tile([P, DK], f32)
        nc.sync.dma_start(out=b0_sb, in_=b0.rearrange("(m p) -> p m", p=P))
        g1_sb = consts.tile([P, DK], f32)
        nc.sync.dma_start(out=g1_sb, in_=g1.rearrange("(m p) -> p m", p=P))
        b1n_sb = consts.tile([P, DK], f32)
        nc.sync.dma_start(out=b1n_sb, in_=b1n.rearrange("(m p) -> p m", p=P))
        b2_sb = consts.tile([P, DK], f32)
        nc.sync.dma_start(out=b2_sb, in_=b2.rearrange("(m p) -> p m", p=P))
        b1_sb = consts.tile([P, HM], f32)
        nc.sync.dma_start(out=b1_sb, in_=b1.rearrange("(m p) -> p m", p=P))
        w2w1n_sb = consts.tile([P, DK], f32)
        nc.sync.dma_start(out=w2w1n_sb, in_=w2w1n.rearrange("(m p) -> p m", p=P))

        # ---- load xT ---------------------------------------------------
        xT_sb = xq.tile([P, DK, TOK], f32)
        xT_re = xT.rearrange("(k p) f -> p k f", p=P)
        xTb = xq.tile([P, DK, TOK], bf16)
        for k in range(DK):
            nc.sync.dma_start(out=xT_sb[:, k, :], in_=xT_re[:, k, :])
            nc.vector.tensor_copy(xTb[:, k, :], xT_sb[:, k, :])

        # ---- K projection first, then its AllGather ------------------
        qT_sb = xq.tile([P, DK, TOK], bf16)
        kT_sb = mid.tile([P, DK, TOK], f8, tag="kv8")

        def _proj(wap, dst):
            wre = wap.rearrange("(k p) m -> p k m", p=P)
            for m in range(DK):
                wt = wst.tile([P, DK, P], bf16, tag="w", name=f"wt_{m}")
                nc.sync.dma_start(out=wt, in_=wre[:, :, m * P : (m + 1) * P])
                pt = ps.tile([P, TOK], f32, tag="pb", name=f"pt_{m}")
                for k in range(DK):
                    nc.tensor.matmul(
                        pt,
                        wt[:, k, :],
                        xTb[:, k, :],
                        start=(k == 0),
                        stop=(k == DK - 1),
                    )
                nc.vector.tensor_copy(dst[:, m, :], pt)

        _proj(WkT, kT_sb)
        for c in range(KCH):
            nc.sync.dma_start(
                out=kv_in_k[c][:].rearrange("(k p f) -> p k f", p=P, k=DK),
                in_=kT_sb[:, :, c * CTOK : (c + 1) * CTOK],
            )
            nc.gpsimd.collective_compute(
                "AllGather",
                mybir.AluOpType.bypass,
                replica_groups=[list(range(NCORES))],
                ins=[kv_in_k[c][:]],
                outs=[kv_out_k[c][:]],
            )

        # ---- V projection, then its AllGather -------------------------
        v_sb = mid.tile([P, MT, D], f8, tag="kv8")
        wvre = WvT.rearrange("(k p) m -> p k m", p=P)
        for n2 in range(2):
            wvt = wv_st.tile([P, DK, TOK], bf16, tag="wv")
            nc.sync.dma_start(out=wvt, in_=wvre[:, :, n2 * TOK : (n2 + 1) * TOK])
            for t in range(MT):
                pt = ps.tile([P, TOK], f32, tag="pb")
                for k in range(DK):
                    nc.tensor.matmul(
                        pt,
                        xTb[:, k, t * P : (t + 1) * P],
                        wvt[:, k, :],
                        start=(k == 0),
                        stop=(k == DK - 1),
                    )
                nc.vector.tensor_add(
                    v_sb[:, t, n2 * TOK : (n2 + 1) * TOK],
                    pt,
                    bv_b[:, n2 * TOK : (n2 + 1) * TOK],
                )
        # V stored [m][p][t][f] per feature-chunk so attention-phase reads
        # of a dout block are contiguous per partition; each chunk's
        # AllGather is issued as soon as its half of V is computed.
        MBLK = P * MT * P
        for c in range(VCH):
            for mi in range(DK // VCH):
                m = c * (DK // VCH) + mi
                nc.sync.dma_start(
                    out=kv_in_v[c][mi * MBLK : (mi + 1) * MBLK].rearrange(
                        "(p t f) -> p t f", p=P, t=MT
                    ),
                    in_=v_sb[:, :, m * P : (m + 1) * P],
                )
            nc.gpsimd.collective_compute(
                "AllGather",
                mybir.AluOpType.bypass,
                replica_groups=[list(range(NCORES))],
                ins=[kv_in_v[c][:]],
                outs=[kv_out_v[c][:]],
            )

        _proj(WqT, qT_sb)

        # ---- scores S^T + exp, denominator interleaved ----------------
        # chunk-outer so each token-chunk's matmuls start as soon as its
        # AllGather lands; the next chunk's gather overlaps.
        pT_sb = big.tile([P, NJ, TOK], bf16, tag="big")
        psd = pss.tile([1, TOK], f32, tag="psm")
        CMJ = CTOK // P  # token tiles per chunk
        for c in range(KCH):
            for r in range(NCORES):
                ktr = kvst.tile([P, DK, CTOK], f8, tag="kt")
                nc.sync.dma_start(
                    out=ktr,
                    in_=kv_out_k[c][r * KC : (r + 1) * KC].rearrange(
                        "(k p f) -> p k f", p=P, k=DK
                    ),
                )
                ktb = kvst.tile([P, DK, CTOK], bf16, tag="ktb")
                nc.vector.tensor_copy(ktb, ktr)
                for mj in range(CMJ):
                    kt_i = r * MT + c * CMJ + mj
                    pt = ps.tile([P, TOK], f32, tag="pb")
                    for k in range(DK):
                        nc.tensor.matmul(
                            pt,
                            ktb[:, k, mj * P : (mj + 1) * P],
                            qT_sb[:, k, :],
                            start=(k == 0),
                            stop=(k == DK - 1),
                        )
                    nc.scalar.activation(
                        pT_sb[:, kt_i, :], pt, Exp, bias=0.0, scale=SCALE
                    )
                    nc.tensor.matmul(
                        psd,
                        ones_b,
                        pT_sb[:, kt_i, :],
                        start=(c == 0 and r == 0 and mj == 0),
                        stop=(c == KCH - 1 and r == NCORES - 1 and mj == CMJ - 1),
                    )
        rden = consts.tile([1, TOK], f32)
        nc.vector.reciprocal(rden, psd)
        nc.sync.dma_start(out=ln_dram[0:1, :], in_=rden)
        rden_b = consts.tile([P, TOK], f32)
        nc.gpsimd.dma_start(out=rden_b, in_=ln_dram[0:1, :].to_broadcast([P, TOK]))

        # ---- attention output attnT = V.T @ P^T, + residual -----------
        # LN0 statistics (sum, sum-of-squares over features) are computed
        # incrementally as each residual feature-tile lands; LN0 itself is
        # folded into the FFN1 weights (host pre-scales W1 by g0), so FFN1
        # can start on the raw residual immediately.
        resb = [
            fts.tile([P, TOK], bf16, name=f"resb{m}", tag=f"resb{m}")
            for m in range(DK)
        ]
        psm0 = pss.tile([1, TOK], f32, tag="psm")
        psq0 = pss.tile([1, TOK], f32, tag="psm")
        for m in range(DK):
            pt = ps.tile([P, TOK], f32, tag="pb")
            for r in range(NCORES):
                vc = m // (DK // VCH)
                mi = m % (DK // VCH)
                vt = vtst.tile([P, MT, P], f8, tag="vt")
                nc.sync.dma_start(
                    out=vt,
                    in_=kv_out_v[vc][
                        r * VC + mi * MBLK : r * VC + (mi + 1) * MBLK
                    ].rearrange("(p t f) -> p t f", p=P, t=MT),
                )
                vtb = vtst.tile([P, MT, P], bf16, tag="vtb")
                nc.vector.tensor_copy(vtb, vt)
                for t in range(MT):
                    kt_i = r * MT + t
                    nc.tensor.matmul(
                        pt,
                        vtb[:, t, :],
                        pT_sb[:, kt_i, :],
                        start=(kt_i == 0),
                        stop=(kt_i == NJ - 1),
                    )
            tmp = ev.tile([P, TOK], f32, tag="sq")
            nc.vector.tensor_mul(tmp, pt, rden_b)
            nc.vector.tensor_add(resb[m][:], tmp, xT_sb[:, m, :])
            sq = ev.tile([P, TOK], bf16, tag="sqb")
            nc.vector.tensor_mul(sq, resb[m][:], resb[m][:])
            nc.tensor.matmul(
                psm0, ones_b, resb[m][:], start=(m == 0), stop=(m == DK - 1)
            )
            nc.tensor.matmul(
                psq0, ones_b, sq, start=(m == 0), stop=(m == DK - 1)
            )

        # ---- layernorm finalize (stats already accumulated) -----------
        def t_layernorm(psm, psq, src, dst_tiles, ln_row):
            mu = consts.tile([1, TOK], f32, tag="ln_mu")
            nc.vector.tensor_scalar_mul(mu, psm, 1.0 / D)
            e2 = consts.tile([1, TOK], f32, tag="ln_e2")
            nc.vector.tensor_scalar_mul(e2, psq, 1.0 / D)
            mu2 = consts.tile([1, TOK], f32, tag="ln_mu2")
            nc.vector.tensor_mul(mu2, mu, mu)
            var = consts.tile([1, TOK], f32, tag="ln_var")
            nc.vector.tensor_sub(var, e2, mu2)
            std = consts.tile([1, TOK], f32, tag="ln_std")
            nc.scalar.activation(std, var, Sqrt, bias=eps_sb[:])
            rstd = consts.tile([1, TOK], f32, tag="ln_rstd")
            nc.vector.reciprocal(rstd, std)
            nc.sync.dma_start(out=ln_dram[ln_row : ln_row + 1, :], in_=mu)
            nc.sync.dma_start(out=ln_dram[ln_row + 1 : ln_row + 2, :], in_=rstd)
            mu_b = consts.tile([P, TOK], f32, tag="ln_mub")
            nc.gpsimd.dma_start(
                out=mu_b, in_=ln_dram[ln_row : ln_row + 1, :].to_broadcast([P, TOK])
            )
            rstd_b = consts.tile([P, TOK], f32, tag="ln_rsb")
            nc.gpsimd.dma_start(
                out=rstd_b,
                in_=ln_dram[ln_row + 1 : ln_row + 2, :].to_broadcast([P, TOK]),
            )
            for m in range(DK):
                t1 = ev.tile([P, TOK], f32, tag="sq")
                nc.vector.tensor_sub(t1, src[:, m, :], mu_b)
                nc.vector.tensor_mul(t1, t1, rstd_b)
                dst_tiles(m, t1)

        # LN0 scale factors: rstd and mu*rstd, broadcast along partitions
        mu0 = consts.tile([1, TOK], f32, tag="ln_mu")
        nc.vector.tensor_scalar_mul(mu0, psm0, 1.0 / D)
        e20 = consts.tile([1, TOK], f32, tag="ln_e2")
        nc.vector.tensor_scalar_mul(e20, psq0, 1.0 / D)
        mu20 = consts.tile([1, TOK], f32, tag="ln_mu2")
        nc.vector.tensor_mul(mu20, mu0, mu0)
        var0 = consts.tile([1, TOK], f32, tag="ln_var")
        nc.vector.tensor_sub(var0, e20, mu20)
        std0 = consts.tile([1, TOK], f32, tag="ln_std")
        nc.scalar.activation(std0, var0, Sqrt, bias=eps_sb[:])
        rstd0 = consts.tile([1, TOK], f32, tag="ln_rstd")
        nc.vector.reciprocal(rstd0, std0)
        msr0 = consts.tile([1, TOK], f32, tag="ln_msr")
        nc.vector.tensor_mul(msr0, mu0, rstd0)
        nc.sync.dma_start(out=ln_dram[1:2, :], in_=rstd0)
        nc.sync.dma_start(out=ln_dram[2:3, :], in_=msr0)
        rstd0_b = consts.tile([P, TOK], f32, tag="ln_rsb0")
        nc.gpsimd.dma_start(
            out=rstd0_b, in_=ln_dram[1:2, :].to_broadcast([P, TOK])
        )
        msr0_b = consts.tile([P, TOK], f32, tag="ln_msb0")
        nc.gpsimd.dma_start(out=msr0_b, in_=ln_dram[2:3, :].to_broadcast([P, TOK]))


        # ---- FFN, chunked over hidden dim -----------------------------
        acc = mid.tile([P, DK, TOK], f32r, tag="acc", bufs=1)
        w1re = W1T.rearrange("(k p) m -> p k m", p=P)
        w2re = W2T.rearrange("(k p) m -> p k m", p=P)
        for hc in range(HC):
            f1c = [
                fts.tile(
                    [P, TOK], bf16, name=f"f1c{m}_{hc}", tag=f"f1c{m}", bufs=2
                )
                for m in range(HCK)
            ]
            for m in range(HCK):
                hm = hc * HCK + m
                w1t = wst.tile([P, DK, P], bf16, tag="w")
                nc.sync.dma_start(out=w1t, in_=w1re[:, :, hm * P : (hm + 1) * P])
                pt = ps.tile([P, TOK], f32, tag="pb")
                for k in range(DK):
                    nc.tensor.matmul(
                        pt,
                        w1t[:, k, :],
                        resb[k][:],
                        start=(k == 0),
                        stop=(k == DK - 1),
                    )
                nc.vector.tensor_copy(f1c[m][:], pt)
            for m in range(DK):
                w2t = wst.tile([P, HCK, P], bf16, tag="w")
                nc.sync.dma_start(
                    out=w2t,
                    in_=w2re[:, hc * HCK : (hc + 1) * HCK, m * P : (m + 1) * P],
                )
                pt = ps.tile([P, TOK], f32, tag="pb")
                for k in range(HCK):
                    nc.tensor.matmul(
                        pt,
                        w2t[:, k, :],
                        f1c[k][:],
                        start=(k == 0),
                        stop=(k == HCK - 1),
                    )
                if hc == 0:
                    nc.vector.tensor_copy(acc[:, m, :], pt)
                else:
                    nc.vector.tensor_add(acc[:, m, :], acc[:, m, :], pt)

        # h = g0*(res - mu)*rstd + b0 for the final residual (off the PE
        # critical path; only needed at the finalize below).
        hT = [
            fts.tile([P, TOK], bf16, name=f"hT{m}", tag=f"hT{m}") for m in range(DK)
        ]
        for m in range(DK):
            t1 = ev.tile([P, TOK], f32, tag="sq")
            nc.vector.tensor_mul(t1, resb[m][:], rstd0_b)
            nc.vector.tensor_sub(t1, t1, msr0_b)
            nc.vector.tensor_scalar(
                hT[m][:],
                t1,
                g0_sb[:, m : m + 1],
                b0_sb[:, m : m + 1],
                op0=mult,
                op1=add,
            )

        # finalize: acc = acc*rstd (deferred LN0 scale, factored out of the
        # hidden-dim sum) + (b2 + W2@c1) - (W2@w1gs)*msr + hT, LN1 stats
        # inline.
        psm1 = pss.tile([1, TOK], f32, tag="psm")
        psq1 = pss.tile([1, TOK], f32, tag="psm")
        for m in range(DK):
            cfix = ev.tile([P, TOK], f32, tag="sq")
            nc.vector.tensor_scalar(
                cfix,
                msr0_b,
                w2w1n_sb[:, m : m + 1],
                b2_sb[:, m : m + 1],
                op0=mult,
                op1=add,
            )
            nc.vector.tensor_mul(acc[:, m, :], acc[:, m, :], rstd0_b)
            nc.vector.tensor_add(acc[:, m, :], acc[:, m, :], cfix)
            nc.vector.tensor_add(acc[:, m, :], acc[:, m, :], hT[m][:])
            sq = ev.tile([P, TOK], f32r, tag="sq")
            nc.vector.tensor_mul(sq, acc[:, m, :], acc[:, m, :])
            nc.tensor.matmul(
                psm1, ones_f, acc[:, m, :], start=(m == 0), stop=(m == DK - 1)
            )
            nc.tensor.matmul(
                psq1, ones_f, sq, start=(m == 0), stop=(m == DK - 1)
            )

        # ---- final layernorm + writeback ------------------------------
        out_re = outT.rearrange("(m p) f -> p m f", p=P)

        def ln1_out(m, t1):
            ot = ev.tile([P, TOK], f32, tag="ot")
            nc.vector.tensor_scalar(
                ot,
                t1,
                g1_sb[:, m : m + 1],
                b1n_sb[:, m : m + 1],
                op0=mult,
                op1=add,
            )
            nc.sync.dma_start(out=out_re[:, m, :], in_=ot)

        t_layernorm(psm1, psq1, acc, ln1_out, 3)

    nc.finalize()
    return nc


def _get_nc():
    if "nc" not in _cache:
        _cache["nc"] = _build_nc()
    return _cache["nc"]


def _make_in_maps(inputs):
    import ml_dtypes

    bf = ml_dtypes.bfloat16
    x = np.ascontiguousarray(np.asarray(inputs["x"], dtype=np.float32))
    shared = {
        "WqT": np.ascontiguousarray(np.asarray(inputs["Wq"], np.float32).T.astype(bf)),
        "WkT": np.ascontiguousarray(np.asarray(inputs["Wk"], np.float32).T.astype(bf)),
        "WvT": np.ascontiguousarray(np.asarray(inputs["Wv"], np.float32).T.astype(bf)),
        "W1T": None,  # filled below (g0-scaled)
        "W2T": np.ascontiguousarray(np.asarray(inputs["W2"], np.float32).T.astype(bf)),
        "bv": np.ascontiguousarray(np.asarray(inputs["bv"], np.float32)),
        "g0": np.ascontiguousarray(np.asarray(inputs["g0"], np.float32)),
        "b0": np.ascontiguousarray(np.asarray(inputs["b0"], np.float32)),
        "b1": np.ascontiguousarray(np.asarray(inputs["b1"], np.float32)),
        "b2": np.ascontiguousarray(np.asarray(inputs["b2"], np.float32)),
        "g1": np.ascontiguousarray(np.asarray(inputs["g1"], np.float32)),
        "b1n": np.ascontiguousarray(np.asarray(inputs["b1n"], np.float32)),
    }
    # LN0 is folded into FFN1: W1' = W1*g0 (per input feature), the bias
    # correction c1 = W1@b0 + b1 rides in the b1 slot, and w1gsn carries
    # -sum_d(W1*g0) for the per-token mean correction.
    W1 = np.asarray(inputs["W1"], np.float64)
    W2 = np.asarray(inputs["W2"], np.float64)
    g0f = np.asarray(inputs["g0"], np.float64)
    b0f = np.asarray(inputs["b0"], np.float64)
    b1f = np.asarray(inputs["b1"], np.float64)
    b2f = np.asarray(inputs["b2"], np.float64)
    W1g = W1 * g0f[None, :]
    c1 = W1 @ b0f + b1f
    shared["W1T"] = np.ascontiguousarray(W1g.T.astype(np.float32).astype(bf))
    shared["b2"] = np.ascontiguousarray((b2f + W2 @ c1).astype(np.float32))
    shared["w2w1n"] = np.ascontiguousarray((-(W2 @ W1g.sum(axis=1))).astype(np.float32))
    in_maps = []
    for c in range(NCORES):
        m = dict(shared)
        m["xT"] = np.ascontiguousarray(x[c * TOK : (c + 1) * TOK, :].T)
        in_maps.append(m)
    return in_maps


def _assemble(res):
    out = np.empty((N, D), dtype=np.float32)
    for c in range(NCORES):
        out[c * TOK : (c + 1) * TOK, :] = res.results[c]["outT"].T
    return out


def kernel(**inputs):
    from concourse import bass_utils

    nc = _get_nc()
    res = bass_utils.run_bass_kernel_spmd(
        nc, _make_in_maps(inputs), core_ids=list(range(NCORES)), trace=False
    )
    return _assemble(res)


def run_traced(inputs):
    """Like kernel() but with NTFF tracing; returns (out, exec_time_ns, results)."""
    import hookshim

    hookshim.install()
    from concourse import bass_utils

    nc = _get_nc()
    res = bass_utils.run_bass_kernel_spmd(
        nc, _make_in_maps(inputs), core_ids=list(range(NCORES)), trace=True
    )
    return _assemble(res), res.exec_time_ns, res



# revision 5
# speedup vs baseline: 1.6099x; 1.6099x over previous
"""Distributed single-head transformer block on 8 TRN2 NeuronCores (v2).

Sharding: token dim (4096) split 8 ways (512 tokens/core), weights
replicated (host pre-transposes so every matmul contracts over the
partition axis). Attention needs all tokens' K/V: each core computes its
local K^T/V scaled by 16 in fp8, and chunked AllGathers distribute them
while the PE keeps computing.

Key numeric/layout tricks vs v1:
  - All attention-path matmuls run fp8 DoubleRow (2x PE throughput):
    QKV projections, scores, attn@V. Gathered K/V are consumed as fp8
    directly (no bf16 casts). Probs are stored fp8 straight out of the
    Exp activation (scale folds the 16x16 weight scaling).
  - The FFN has NO activation between its two Linears, so it collapses
    into one [D,D] matmul precomputed on the host:
        Mi = W2@W1 + I;  out_pre = h@Mi.T + (W2@b1 + b2)
    with LN0 folded in:  out_pre = rstd0*(Wff@res) + msr0*wcol + cb,
        Wff = Mi*g0[None,:], wcol = -Mi@g0, cb = Mi@b0 + W2@b1 + b2.
    This cuts FFN PE work 8x and FFN weight DMA to 2 MB.
  - Per-token scalars (1/denom, rstd, mu*rstd) are broadcast along
    partitions with a rank-1 PE matmul into PSUM instead of a DRAM
    roundtrip.
  - LN sums use 1/D-valued ones vectors so the stats matmuls produce
    means directly.

Output is outT [D, 512] per core; the host transposes and concatenates.
"""

import numpy as np

P = 128
D = 1024
N = 4096
NCORES = 8
TOK = N // NCORES  # 512 tokens per core
DK = D // P  # 8   feature tiles
MT = TOK // P  # 4   local token tiles
NJ = N // P  # 32  global key tiles
SCALE = 1.0 / float(np.sqrt(D))
WS = 16.0  # fp8 scale for QKV weights (and thus q/k/v activations)
EPS = 1e-5

KSPLIT = [2, 2]  # K AllGather chunks, in token tiles per rank
VSPLIT = [4, 3, 1]  # V AllGather chunks, in feature tiles
MBLK = P * MT * P  # one V feature-tile for one rank, elements

_cache = {}


def _build_nc():
    import concourse.tile as tile
    from concourse import bacc, mybir
    from contextlib import ExitStack

    f32 = mybir.dt.float32
    f32r = mybir.dt.float32r
    bf16 = mybir.dt.bfloat16
    f8 = mybir.dt.float8e4
    Exp = mybir.ActivationFunctionType.Exp
    Sqrt = mybir.ActivationFunctionType.Sqrt
    Ident = mybir.ActivationFunctionType.Identity
    DR = mybir.MatmulPerfMode.DoubleRow

    nc = bacc.Bacc("TRN2", target_bir_lowering=False, debug=False, num_devices=NCORES)

    xT = nc.dram_tensor("xT", [D, TOK], f32, kind="ExternalInput").ap()
    WqT = nc.dram_tensor("WqT", [D, D], f8, kind="ExternalInput").ap()
    WkT = nc.dram_tensor("WkT", [D, D], f8, kind="ExternalInput").ap()
    WvT = nc.dram_tensor("WvT", [D, D], f8, kind="ExternalInput").ap()
    WffT = nc.dram_tensor("WffT", [D, D], bf16, kind="ExternalInput").ap()
    bv16 = nc.dram_tensor("bv16", [D], f32, kind="ExternalInput").ap()
    wcol = nc.dram_tensor("wcol", [D], f32, kind="ExternalInput").ap()
    cbv = nc.dram_tensor("cbv", [D], f32, kind="ExternalInput").ap()
    g1 = nc.dram_tensor("g1", [D], f32, kind="ExternalInput").ap()
    b1n = nc.dram_tensor("b1n", [D], f32, kind="ExternalInput").ap()
    outT = nc.dram_tensor("outT", [D, TOK], f32, kind="ExternalOutput").ap()

    with tile.TileContext(nc) as tc, ExitStack() as ctx:
        dram = ctx.enter_context(tc.tile_pool(name="dram", bufs=1, space="DRAM"))
        consts = ctx.enter_context(tc.tile_pool(name="consts", bufs=1))
        xq = ctx.enter_context(tc.tile_pool(name="xq", bufs=1))
        big = ctx.enter_context(tc.tile_pool(name="big", bufs=1))
        wst = ctx.enter_context(tc.tile_pool(name="wst", bufs=3))
        wv_st = ctx.enter_context(tc.tile_pool(name="wv_st", bufs=2))
        kvst = ctx.enter_context(tc.tile_pool(name="kvst", bufs=2))
        vtst = ctx.enter_context(tc.tile_pool(name="vtst", bufs=3))
        ev = ctx.enter_context(tc.tile_pool(name="ev", bufs=4))
        fts = ctx.enter_context(tc.tile_pool(name="fts", bufs=1))
        bcs = ctx.enter_context(tc.tile_pool(name="bcs", bufs=2))
        lns = ctx.enter_context(tc.tile_pool(name="lns", bufs=4))
        ps = ctx.enter_context(tc.tile_pool(name="ps", bufs=4, space="PSUM"))
        bc = ctx.enter_context(tc.tile_pool(name="bc", bufs=2, space="PSUM"))
        pss = ctx.enter_context(tc.tile_pool(name="pss", bufs=2, space="PSUM"))

        kv_in_k, kv_out_k = [], []
        for c, ctiles in enumerate(KSPLIT):
            csz = ctiles * P
            kv_in_k.append(
                dram.tile([D * csz], f8, name=f"kvik{c}", tag=f"kvik{c}")
            )
            kv_out_k.append(
                dram.tile(
                    [NCORES * D * csz],
                    f8,
                    addr_space="Shared",
                    name=f"kvok{c}",
                    tag=f"kvok{c}",
                )
            )
        kv_in_v, kv_out_v = [], []
        for c, cm in enumerate(VSPLIT):
            kv_in_v.append(
                dram.tile([cm * MBLK], f8, name=f"kviv{c}", tag=f"kviv{c}")
            )
            kv_out_v.append(
                dram.tile(
                    [NCORES * cm * MBLK],
                    f8,
                    addr_space="Shared",
                    name=f"kvov{c}",
                    tag=f"kvov{c}",
                )
            )

        # ---- constants -------------------------------------------------
        stage = consts.tile([P, 1], f32)
        nc.vector.memset(stage, 1.0)
        ones_f8 = consts.tile([P, 1], f8)
        nc.vector.tensor_copy(ones_f8, stage)
        stage2 = consts.tile([P, 1], f32)
        nc.vector.memset(stage2, 1.0 / D)
        onesd_r = consts.tile([P, 1], f32r)
        nc.vector.tensor_copy(onesd_r, stage2)
        onesd_b = consts.tile([P, 1], bf16)
        nc.vector.tensor_copy(onesd_b, stage2)
        ones16_r = consts.tile([1, P], f32)
        nc.vector.memset(ones16_r, 1.0 / WS)
        ones1_r = consts.tile([1, P], f32)
        nc.vector.memset(ones1_r, 1.0)
        eps_sb = consts.tile([1, 1], f32)
        nc.vector.memset(eps_sb, EPS)
        bv_b = consts.tile([P, D], f32)
        nc.gpsimd.dma_start(out=bv_b, in_=bv16[None, :].to_broadcast([P, D]))
        g1_sb = consts.tile([P, DK], f32)
        nc.sync.dma_start(out=g1_sb, in_=g1.rearrange("(m p) -> p m", p=P))
        b1n_sb = consts.tile([P, DK], f32)
        nc.sync.dma_start(out=b1n_sb, in_=b1n.rearrange("(m p) -> p m", p=P))
        wcol_sb = consts.tile([P, DK], f32)
        nc.sync.dma_start(out=wcol_sb, in_=wcol.rearrange("(m p) -> p m", p=P))
        cb_sb = consts.tile([P, DK], f32)
        nc.sync.dma_start(out=cb_sb, in_=cbv.rearrange("(m p) -> p m", p=P))

        # ---- load xT, cast to fp8 --------------------------------------
        xT_sb = xq.tile([P, DK, TOK], f32)
        x_f8 = xq.tile([P, DK, TOK], f8)
        xT_re = xT.rearrange("(k p) f -> p k f", p=P)
        for k in range(DK):
            nc.sync.dma_start(out=xT_sb[:, k, :], in_=xT_re[:, k, :])
            nc.vector.tensor_copy(x_f8[:, k, :], xT_sb[:, k, :])

        # ---- K projection (fp8 DoubleRow), then its AllGather ---------
        kT_f8 = xq.tile([P, DK, TOK], f8)
        qT_f8 = xq.tile([P, DK, TOK], f8)

        def _proj(wap, dst):
            wre = wap.rearrange("(k p) m -> p k m", p=P)
            for m in range(DK):
                wt = wst.tile([P, DK, P], f8, tag="w", name=f"wt_{m}")
                nc.sync.dma_start(out=wt, in_=wre[:, :, m * P : (m + 1) * P])
                pt = ps.tile([P, TOK], f32, tag="pb", name=f"pt_{m}")
                for k2 in range(DK // 2):
                    nc.tensor.matmul(
                        pt,
                        wt[:, 2 * k2 : 2 * k2 + 2, :],
                        x_f8[:, 2 * k2 : 2 * k2 + 2, :],
                        start=(k2 == 0),
                        stop=(k2 == DK // 2 - 1),
                        perf_mode=DR,
                    )
                nc.vector.tensor_copy(dst[:, m, :], pt)

        _proj(WkT, kT_f8)
        kt0 = 0
        for c, ctiles in enumerate(KSPLIT):
            csz = ctiles * P
            nc.sync.dma_start(
                out=kv_in_k[c][:].rearrange("(k p f) -> p k f", p=P, k=DK),
                in_=kT_f8[:, :, kt0 * P : kt0 * P + csz],
            )
            nc.gpsimd.collective_compute(
                "AllGather",
                mybir.AluOpType.bypass,
                replica_groups=[list(range(NCORES))],
                ins=[kv_in_k[c][:]],
                outs=[kv_out_k[c][:]],
            )
            kt0 += ctiles

        # ---- V projection (fp8 DoubleRow), then its AllGather ---------
        v_sb = xq.tile([P, MT, D], f8)
        wvre = WvT.rearrange("(k p) m -> p k m", p=P)
        for n2 in range(2):
            wvt = wv_st.tile([P, DK, TOK], f8, tag="wv")
            nc.sync.dma_start(out=wvt, in_=wvre[:, :, n2 * TOK : (n2 + 1) * TOK])
            for t in range(MT):
                pt = ps.tile([P, TOK], f32, tag="pb")
                for k2 in range(DK // 2):
                    nc.tensor.matmul(
                        pt,
                        x_f8[:, 2 * k2 : 2 * k2 + 2, t * P : (t + 1) * P],
                        wvt[:, 2 * k2 : 2 * k2 + 2, :],
                        start=(k2 == 0),
                        stop=(k2 == DK // 2 - 1),
                        perf_mode=DR,
                    )
                nc.vector.tensor_add(
                    v_sb[:, t, n2 * TOK : (n2 + 1) * TOK],
                    pt,
                    bv_b[:, n2 * TOK : (n2 + 1) * TOK],
                )
        m0 = 0
        for c, cm in enumerate(VSPLIT):
            for mi in range(cm):
                m = m0 + mi
                nc.sync.dma_start(
                    out=kv_in_v[c][mi * MBLK : (mi + 1) * MBLK].rearrange(
                        "(p t f) -> p t f", p=P, t=MT
                    ),
                    in_=v_sb[:, :, m * P : (m + 1) * P],
                )
            nc.gpsimd.collective_compute(
                "AllGather",
                mybir.AluOpType.bypass,
                replica_groups=[list(range(NCORES))],
                ins=[kv_in_v[c][:]],
                outs=[kv_out_v[c][:]],
            )
            m0 += cm

        _proj(WqT, qT_f8)

        # prefetch the collapsed-FFN weights during the attention phase
        wff_sb = fts.tile([P, DK * DK, P], bf16)
        wffre = WffT.rearrange("(k p) m -> p k m", p=P)
        for m in range(DK):
            nc.sync.dma_start(
                out=wff_sb[:, m * DK : (m + 1) * DK, :],
                in_=wffre[:, :, m * P : (m + 1) * P],
            )

        # ---- scores S^T -> exp -> fp8 probs, denominator interleaved --
        pT_sb = big.tile([P, NJ, TOK], f8, tag="big")
        psd = pss.tile([1, TOK], f32, tag="psm")
        tbase = 0
        for c, ctiles in enumerate(KSPLIT):
            csz = ctiles * P
            for r in range(NCORES):
                ktb = kvst.tile([P, DK, csz], f8, tag=f"kt{c}", name=f"ktb{c}_{r}")
                nc.sync.dma_start(
                    out=ktb,
                    in_=kv_out_k[c][r * D * csz : (r + 1) * D * csz].rearrange(
                        "(k p f) -> p k f", p=P, k=DK
                    ),
                )
                for mj in range(ctiles):
                    kt_i = r * MT + tbase + mj
                    pt = ps.tile([P, TOK], f32, tag="pb")
                    for k2 in range(DK // 2):
                        nc.tensor.matmul(
                            pt,
                            ktb[:, 2 * k2 : 2 * k2 + 2, mj * P : (mj + 1) * P],
                            qT_f8[:, 2 * k2 : 2 * k2 + 2, :],
                            start=(k2 == 0),
                            stop=(k2 == DK // 2 - 1),
                            perf_mode=DR,
                        )
                    nc.scalar.activation(
                        pT_sb[:, kt_i, :], pt, Exp, bias=0.0, scale=SCALE / (WS * WS)
                    )
                    nc.tensor.matmul(
                        psd,
                        ones_f8,
                        pT_sb[:, kt_i, :],
                        start=(c == 0 and r == 0 and mj == 0),
                        stop=(
                            c == len(KSPLIT) - 1
                            and r == NCORES - 1
                            and mj == ctiles - 1
                        ),
                    )
            tbase += ctiles

        # rden = 1/(16*denom), broadcast along partitions via rank-1 MM
        rden = lns.tile([1, TOK], f32, tag="ln")
        nc.vector.reciprocal(rden, psd)
        rden_bp = bc.tile([P, TOK], f32, tag="bc")
        nc.tensor.matmul(rden_bp, ones16_r, rden)
        rden_sb = bcs.tile([P, TOK], f32, tag="bcs")
        nc.scalar.copy(rden_sb, rden_bp)

        # ---- attention output attnT = V.T @ P^T (fp8 DR), + residual --
        resb = [
            fts.tile([P, TOK], bf16, name=f"resb{m}", tag=f"resb{m}")
            for m in range(DK)
        ]
        psm0 = pss.tile([1, TOK], f32, tag="psm")
        psq0 = pss.tile([1, TOK], f32, tag="psm")
        m0 = 0
        for c, cm in enumerate(VSPLIT):
            for mi in range(cm):
                m = m0 + mi
                pt = ps.tile([P, TOK], f32, tag="pb")
                for r in range(NCORES):
                    vt = vtst.tile([P, MT, P], f8, tag="vt")
                    nc.sync.dma_start(
                        out=vt,
                        in_=kv_out_v[c][
                            (r * cm + mi) * MBLK : (r * cm + mi + 1) * MBLK
                        ].rearrange("(p t f) -> p t f", p=P, t=MT),
                    )
                    for tp in range(MT // 2):
                        kt_i = r * MT + 2 * tp
                        nc.tensor.matmul(
                            pt,
                            vt[:, 2 * tp : 2 * tp + 2, :],
                            pT_sb[:, kt_i : kt_i + 2, :],
                            start=(r == 0 and tp == 0),
                            stop=(r == NCORES - 1 and tp == MT // 2 - 1),
                            perf_mode=DR,
                        )
                tmp = ev.tile([P, TOK], f32, tag="ev")
                nc.vector.tensor_mul(tmp, pt, rden_sb)
                nc.vector.tensor_add(resb[m][:], tmp, xT_sb[:, m, :])
                sq = ev.tile([P, TOK], bf16, tag="evb")
                nc.vector.tensor_mul(sq, resb[m][:], resb[m][:])
                nc.tensor.matmul(
                    psm0, onesd_b, resb[m][:], start=(m == 0), stop=(m == DK - 1)
                )
                nc.tensor.matmul(
                    psq0, onesd_b, sq, start=(m == 0), stop=(m == DK - 1)
                )
            m0 += cm

        # ---- LN0 stats finalize: rstd0 / mu0*rstd0 broadcasts ---------
        def ln_chain(psm, psq):
            mu2 = lns.tile([1, TOK], f32, tag="ln")
            nc.scalar.square(mu2, psm)
            var = lns.tile([1, TOK], f32, tag="ln")
            nc.vector.tensor_sub(var, psq, mu2)
            std = lns.tile([1, TOK], f32, tag="ln")
            nc.scalar.activation(std, var, Sqrt, bias=eps_sb[:])
            rstd = lns.tile([1, TOK], f32, tag="ln")
            nc.vector.reciprocal(rstd, std)
            msr = lns.tile([1, TOK], f32, tag="ln")
            nc.vector.tensor_mul(msr, psm, rstd)
            rstd_bp = bc.tile([P, TOK], f32, tag="bc")
            nc.tensor.matmul(rstd_bp, ones1_r, rstd)
            msr_bp = bc.tile([P, TOK], f32, tag="bc")
            nc.tensor.matmul(msr_bp, ones1_r, msr)
            return rstd_bp, msr_bp

        rstd0_bp, msr0_bp = ln_chain(psm0, psq0)
        rstd0_sb = bcs.tile([P, TOK], f32, tag="bcs")
        nc.scalar.copy(rstd0_sb, rstd0_bp)

        # ---- collapsed FFN + finalize + LN1 stats ---------------------
        out1 = xq.tile([P, DK, TOK], f32r)
        psm1 = pss.tile([1, TOK], f32, tag="psm")
        psq1 = pss.tile([1, TOK], f32, tag="psm")
        for m in range(DK):
            pt = ps.tile([P, TOK], f32, tag="pb")
            for k in range(DK):
                nc.tensor.matmul(
                    pt,
                    wff_sb[:, m * DK + k, :],
                    resb[k][:],
                    start=(k == 0),
                    stop=(k == DK - 1),
                )
            u = ev.tile([P, TOK], f32, tag="ev")
            nc.scalar.activation(
                u,
                msr0_bp,
                Ident,
                bias=cb_sb[:, m : m + 1],
                scale=wcol_sb[:, m : m + 1],
            )
            t1 = ev.tile([P, TOK], f32, tag="ev")
            nc.vector.tensor_mul(t1, pt, rstd0_sb)
            nc.vector.tensor_add(out1[:, m, :], t1, u)
            sq = ev.tile([P, TOK], bf16, tag="evb")
            nc.vector.tensor_mul(sq, out1[:, m, :], out1[:, m, :])
            nc.tensor.matmul(
                psm1, onesd_r, out1[:, m, :], start=(m == 0), stop=(m == DK - 1)
            )
            nc.tensor.matmul(
                psq1, onesd_b, sq, start=(m == 0), stop=(m == DK - 1)
            )

        # ---- final layernorm + writeback ------------------------------
        rstd1_bp, msr1_bp = ln_chain(psm1, psq1)
        out_re = outT.rearrange("(m p) f -> p m f", p=P)
        for m in range(DK):
            t1 = ev.tile([P, TOK], f32, tag="ev")
            nc.vector.tensor_mul(t1, out1[:, m, :], rstd1_bp)
            t2 = ev.tile([P, TOK], f32, tag="ev")
            nc.vector.tensor_sub(t2, t1, msr1_bp)
            ot = ev.tile([P, TOK], f32, tag="ev")
            nc.scalar.activation(
                ot,
                t2,
                Ident,
                bias=b1n_sb[:, m : m + 1],
                scale=g1_sb[:, m : m + 1],
            )
            nc.sync.dma_start(out=out_re[:, m, :], in_=ot)

    nc.finalize()
    return nc


def _get_nc():
    if "nc" not in _cache:
        _cache["nc"] = _build_nc()
    return _cache["nc"]


def _make_in_maps(inputs):
    import ml_dtypes

    bf = ml_dtypes.bfloat16
    f8 = ml_dtypes.float8_e4m3
    x = np.ascontiguousarray(np.asarray(inputs["x"], dtype=np.float32))
    Wq = np.asarray(inputs["Wq"], np.float64)
    Wk = np.asarray(inputs["Wk"], np.float64)
    Wv = np.asarray(inputs["Wv"], np.float64)
    W1 = np.asarray(inputs["W1"], np.float64)
    W2 = np.asarray(inputs["W2"], np.float64)
    g0 = np.asarray(inputs["g0"], np.float64)
    b0 = np.asarray(inputs["b0"], np.float64)
    b1 = np.asarray(inputs["b1"], np.float64)
    b2 = np.asarray(inputs["b2"], np.float64)
    bv = np.asarray(inputs["bv"], np.float64)
    # FFN has no activation between the Linears: collapse + LN0 fold.
    Mi = W2 @ W1 + np.eye(D)
    Wff = Mi * g0[None, :]

    def to_f8(a):
        return np.ascontiguousarray(
            np.clip(a, -240.0, 240.0).astype(np.float32)
        ).astype(f8)

    shared = {
        "WqT": to_f8((WS * Wq).T),
        "WkT": to_f8((WS * Wk).T),
        "WvT": to_f8((WS * Wv).T),
        "WffT": np.ascontiguousarray(Wff.T.astype(np.float32)).astype(bf),
        "bv16": np.ascontiguousarray((WS * bv).astype(np.float32)),
        "wcol": np.ascontiguousarray((-(Mi @ g0)).astype(np.float32)),
        "cbv": np.ascontiguousarray((Mi @ b0 + W2 @ b1 + b2).astype(np.float32)),
        "g1": np.ascontiguousarray(np.asarray(inputs["g1"], np.float32)),
        "b1n": np.ascontiguousarray(np.asarray(inputs["b1n"], np.float32)),
    }
    in_maps = []
    for c in range(NCORES):
        m = dict(shared)
        m["xT"] = np.ascontiguousarray(x[c * TOK : (c + 1) * TOK, :].T)
        in_maps.append(m)
    return in_maps


def _assemble(res):
    out = np.empty((N, D), dtype=np.float32)
    for c in range(NCORES):
        out[c * TOK : (c + 1) * TOK, :] = res.results[c]["outT"].T
    return out


def kernel(**inputs):
    from concourse import bass_utils

    nc = _get_nc()
    res = bass_utils.run_bass_kernel_spmd(
        nc, _make_in_maps(inputs), core_ids=list(range(NCORES)), trace=False
    )
    return _assemble(res)


def run_traced(inputs):
    """Like kernel() but with NTFF tracing; returns (out, exec_time_ns, results)."""
    import hookshim

    hookshim.install()
    from concourse import bass_utils

    nc = _get_nc()
    res = bass_utils.run_bass_kernel_spmd(
        nc, _make_in_maps(inputs), core_ids=list(range(NCORES)), trace=True
    )
    return _assemble(res), res.exec_time_ns, res
